# revision 1
# baseline (speedup 1.0000x reference)
"""Two-layer GCN (GCNConv -> ReLU -> GCNConv -> softmax) on 8 Trainium2 NeuronCores.

Sharding: nodes (rows of x, degrees, output) are split 2048-per-core; edges are
partitioned by destination node so each core owns the scatter for its node
shard; the small weight matrices are replicated.  Source-node features are
exchanged with an AllGather between layers.

Per-core aggregation: edges are bucketed by 128-node destination tile.  For
each tile, gathered source rows (dma_gather) are combined with a one-hot
matrix built on the vector engine (iota == dst_rel) and reduced on the tensor
engine via PSUM-accumulated matmuls; the self-loop term is an identity-matmul
chunk.  deg^-1/2 scaling, bias, relu and the row softmax run on DVE/ACT.
"""
import numpy as np

N = 16384
NC = 8
NSH = N // NC        # 2048 nodes per core
TPC = NSH // 128     # 16 destination tiles per core
C_IN, C_HID, C_OUT = 128, 64, 16

_CACHE = {}


def _build_program(CH, stage=4, reps=1, nocc=False, ablate=()):
    import concourse.bacc as bacc
    import concourse.bass as bass
    import concourse.mybir as mybir
    import concourse.tile as tile
    from concourse import library_config
    from contextlib import ExitStack

    f32 = mybir.dt.float32
    i16 = mybir.dt.int16
    NIDX = CH * 128

    nc = bacc.Bacc("TRN2", target_bir_lowering=False, debug=False,
                   enable_asserts=False, num_devices=NC)

    # -------- I/O --------
    d_xT = nc.dram_tensor("xT", [C_IN, NSH], f32, kind="ExternalInput").ap()
    d_w1 = nc.dram_tensor("w1", [C_IN, C_HID], f32, kind="ExternalInput").ap()
    d_w2 = nc.dram_tensor("w2", [C_HID, C_OUT], f32, kind="ExternalInput").ap()
    d_b1 = nc.dram_tensor("b1b", [128, C_HID], f32, kind="ExternalInput").ap()
    d_b2 = nc.dram_tensor("b2b", [128, C_OUT], f32, kind="ExternalInput").ap()
    d_deg = nc.dram_tensor("degp", [128, TPC], f32, kind="ExternalInput").ap()
    d_iota = nc.dram_tensor("iota", [128, 128], f32, kind="ExternalInput").ap()
    d_id = nc.dram_tensor("ident", [128, 128], f32, kind="ExternalInput").ap()
    d_esrc = nc.dram_tensor("esrc", [128, TPC * CH * 8], i16,
                            kind="ExternalInput").ap()
    d_drel = nc.dram_tensor("drel", [128, TPC * CH], f32,
                            kind="ExternalInput").ap()
    d_out = nc.dram_tensor("out", [NSH, C_OUT], f32, kind="ExternalOutput").ap()

    # -------- internal DRAM (collectives) --------
    ht1_sh = nc.dram_tensor("ht1_sh", [NSH, C_HID], f32).ap()
    ht1_full = nc.dram_tensor("ht1_full", [N, C_HID], f32,
                              addr_space="Shared").ap()
    ht2_sh = nc.dram_tensor("ht2_sh", [NSH, C_HID], f32).ap()
    ht2_full = nc.dram_tensor("ht2_full", [N, C_HID], f32,
                              addr_space="Shared").ap()

    rg = [list(range(NC))]

    with tile.TileContext(nc) as tc, ExitStack() as ctx:
        cp = ctx.enter_context(tc.tile_pool(name="const", bufs=1))
        gp = ctx.enter_context(tc.tile_pool(name="gather", bufs=3))
        qp = ctx.enter_context(tc.tile_pool(name="onehot", bufs=3))
        wp = ctx.enter_context(tc.tile_pool(name="work", bufs=3))
        pp = ctx.enter_context(tc.tile_pool(name="psum", bufs=3, space="PSUM"))
        pp2 = ctx.enter_context(tc.tile_pool(name="psum2", bufs=2, space="PSUM"))

        nc.gpsimd.load_library(library_config.mlp)

        # ---- persistent SBUF ----
        sb_xT = cp.tile([C_IN, NSH], f32)
        sb_w1 = cp.tile([C_IN, C_HID], f32)
        sb_w2 = cp.tile([C_HID, C_OUT], f32)
        sb_b1 = cp.tile([128, C_HID], f32)
        sb_b2 = cp.tile([128, C_OUT], f32)
        sb_deg = cp.tile([128, TPC], f32)
        sb_iota = cp.tile([128, 128], f32)
        sb_id = cp.tile([128, 128], f32)
        sb_esrc = cp.tile([128, TPC * CH * 8], i16)
        sb_drel = cp.tile([128, TPC * CH], f32)
        sb_ht1 = cp.tile([128, TPC * C_HID], f32)    # is * (x@W1), own shard
        sb_out1 = cp.tile([128, TPC * C_HID], f32)   # relu'd layer-1 out
        sb_ht2p = cp.tile([128, TPC * C_HID], f32)   # is * (out1@W2), padded
        sb_out2 = cp.tile([128, TPC * C_OUT], f32)

        for dst, src in ((sb_xT, d_xT), (sb_w1, d_w1), (sb_w2, d_w2),
                         (sb_b1, d_b1), (sb_b2, d_b2), (sb_deg, d_deg),
                         (sb_iota, d_iota), (sb_id, d_id), (sb_esrc, d_esrc),
                         (sb_drel, d_drel)):
            nc.sync.dma_start(dst[:], src[:])


        # ---- inv_sqrt(deg) ----
        nc.vector.memset(sb_ht2p[:], 0.0)
        sb_sq = cp.tile([128, TPC], f32)
        sb_is = cp.tile([128, TPC], f32)
        for _rep in range(reps):
          nc.scalar.sqrt(sb_sq[:], sb_deg[:])
          nc.vector.reciprocal(sb_is[:], sb_sq[:])

          # ---- ht1 = is * (x @ W1) ----
          for t in range(TPC):
              psu = pp.tile([128, C_HID], f32, tag="psA")
              nc.tensor.matmul(psu[:], sb_xT[:, t * 128:(t + 1) * 128], sb_w1[:],
                               start=True, stop=True)
              nc.vector.tensor_scalar_mul(
                  sb_ht1[:, t * C_HID:(t + 1) * C_HID], psu[:],
                  sb_is[:, t:t + 1])
          nc.sync.dma_start(
              ht1_sh.rearrange("(t p) c -> p t c", p=128),
              sb_ht1[:].rearrange("p (t c) -> p t c", t=TPC))
          if nocc:
              nc.sync.dma_start(ht1_full[0:NSH, :], ht1_sh[:])
          else:
              nc.gpsimd.collective_compute(
                  "AllGather", mybir.AluOpType.bypass, replica_groups=rg,
                  ins=[ht1_sh[:]], outs=[ht1_full[:]])

          # ---- layer 1 aggregation + fused u2 = out1 @ W2 ----
          for t in range(TPC if stage >= 1 else 0):
              G = gp.tile([128, CH, C_HID], f32, tag="G")
              if "noG" in ablate:
                  nc.sync.dma_start(G[:].rearrange("p c d -> p (c d)")[:, 0:64], ht1_full[0:128, :])
              else:
                  nc.gpsimd.dma_gather(
                      G[:], ht1_full[:],
                      sb_esrc[:, t * CH * 8:(t + 1) * CH * 8], NIDX, NIDX, C_HID,
                      single_packet=False)
              Q = qp.tile([128, CH * 128], f32, tag="Q")
              if "noQ" in ablate:
                  nc.vector.memset(Q[:, 0:128], 0.0)
              else:
                  nc.vector.tensor_tensor(
                  Q[:].rearrange("p (c d) -> p c d", c=CH),
                  sb_drel[:, t * CH:(t + 1) * CH].unsqueeze(2)
                      .broadcast_to([128, CH, 128]),
                  sb_iota[:].unsqueeze(1).broadcast_to([128, CH, 128]),
                  mybir.AluOpType.is_equal)
              ps = pp.tile([128, C_HID], f32, tag="psA")
              for c in range(1 if "noMM" in ablate else CH):
                  nc.tensor.matmul(ps[:], Q[:, c * 128:(c + 1) * 128],
                                   G[:, c, :], start=(c == 0), stop=False)
              nc.tensor.matmul(ps[:], sb_id[:],
                               sb_ht1[:, t * C_HID:(t + 1) * C_HID],
                               start=False, stop=True)
              o1 = sb_out1[:, t * C_HID:(t + 1) * C_HID]
              nc.vector.scalar_tensor_tensor(
                  o1, ps[:], sb_is[:, t:t + 1], sb_b1[:],
                  mybir.AluOpType.mult, mybir.AluOpType.add)
              nc.scalar.activation(o1, o1, mybir.ActivationFunctionType.Relu)

              # u2 tile: transpose out1 tile, matmul with W2, scale by is
              pstr = pp2.tile([C_HID, 128], f32, tag="psT")
              nc.tensor.transpose(pstr[:], o1, sb_id[:])
              o1T = wp.tile([C_HID, 128], f32, tag="o1T")
              nc.vector.tensor_copy(o1T[:], pstr[:])
              psu2 = pp2.tile([128, C_OUT], f32, tag="psU")
              nc.tensor.matmul(psu2[:], o1T[:], sb_w2[:], start=True, stop=True)
              nc.vector.tensor_scalar_mul(
                  sb_ht2p[:, t * C_HID:t * C_HID + C_OUT], psu2[:],
                  sb_is[:, t:t + 1])

          if stage >= 2:
              nc.sync.dma_start(
                  ht2_sh.rearrange("(t p) c -> p t c", p=128),
                  sb_ht2p[:].rearrange("p (t c) -> p t c", t=TPC))
              if nocc:
                  nc.sync.dma_start(ht2_full[0:NSH, :], ht2_sh[:])
              else:
                  nc.gpsimd.collective_compute(
                      "AllGather", mybir.AluOpType.bypass, replica_groups=rg,
                      ins=[ht2_sh[:]], outs=[ht2_full[:]])

          # ---- layer 2 aggregation + softmax ----
          nc.vector.memset(sb_out2[:], 0.0)
          for t in range(TPC if stage >= 3 else 0):
              G = gp.tile([128, CH, C_HID], f32, tag="G")
              nc.gpsimd.dma_gather(
                  G[:], ht2_full[:],
                  sb_esrc[:, t * CH * 8:(t + 1) * CH * 8], NIDX, NIDX, C_HID,
                  single_packet=False)
              Q = qp.tile([128, CH * 128], f32, tag="Q")
              nc.vector.tensor_tensor(
                  Q[:].rearrange("p (c d) -> p c d", c=CH),
                  sb_drel[:, t * CH:(t + 1) * CH].unsqueeze(2)
                      .broadcast_to([128, CH, 128]),
                  sb_iota[:].unsqueeze(1).broadcast_to([128, CH, 128]),
                  mybir.AluOpType.is_equal)
              ps = pp.tile([128, C_OUT], f32, tag="psA")
              for c in range(CH):
                  nc.tensor.matmul(ps[:], Q[:, c * 128:(c + 1) * 128],
                                   G[:, c, 0:C_OUT], start=(c == 0), stop=False)
              nc.tensor.matmul(ps[:], sb_id[:],
                               sb_ht2p[:, t * C_HID:t * C_HID + C_OUT],
                               start=False, stop=True)
              z = wp.tile([128, C_OUT], f32, tag="z")
              nc.vector.scalar_tensor_tensor(
                  z[:], ps[:], sb_is[:, t:t + 1], sb_b2[:],
                  mybir.AluOpType.mult, mybir.AluOpType.add)
              # softmax along the 16 columns
              negm = wp.tile([128, 1], f32, tag="negm")
              nc.vector.tensor_reduce(negm[:], z[:], mybir.AxisListType.X,
                                      mybir.AluOpType.max, negate=True)
              e = sb_out2[:, t * C_OUT:(t + 1) * C_OUT]
              nc.scalar.activation(e, z[:], mybir.ActivationFunctionType.Exp,
                                   bias=negm[:, 0:1], scale=1.0)
              ssum = wp.tile([128, 1], f32, tag="ssum")
              nc.vector.tensor_reduce(ssum[:], e, mybir.AxisListType.X,
                                      mybir.AluOpType.add)
              rcp = wp.tile([128, 1], f32, tag="rcp")
              nc.vector.reciprocal(rcp[:], ssum[:])
              nc.vector.tensor_scalar_mul(e, e, rcp[:, 0:1])

          nc.sync.dma_start(
              d_out.rearrange("(t p) c -> p t c", p=128),
              sb_out2[:].rearrange("p (t c) -> p t c", t=TPC))

    nc.compile()
    return nc


def _host_prep(x, edge_index, W1, b1, W2, b2):
    src = np.asarray(edge_index[0]).astype(np.int64)
    dst = np.asarray(edge_index[1]).astype(np.int64)
    x = np.asarray(x, dtype=np.float32)

    deg1 = (np.bincount(dst, minlength=N) + 1).astype(np.float32)
    tile_id = dst >> 7
    order = np.argsort(tile_id, kind="stable")
    s_src = src[order]
    s_tile = tile_id[order]
    counts = np.bincount(s_tile, minlength=128)
    CH = int(np.ceil(counts.max() / 128))
    NIDX = CH * 128
    starts = np.zeros(129, dtype=np.int64)
    np.cumsum(counts, out=starts[1:])
    s_dst = dst[order]

    iota = np.ascontiguousarray(
        np.tile(np.arange(128, dtype=np.float32), (128, 1)))
    ident = np.eye(128, dtype=np.float32)
    b1b = np.ascontiguousarray(np.tile(np.asarray(b1, np.float32), (128, 1)))
    b2b = np.ascontiguousarray(np.tile(np.asarray(b2, np.float32), (128, 1)))
    w1 = np.ascontiguousarray(np.asarray(W1, np.float32))
    w2 = np.ascontiguousarray(np.asarray(W2, np.float32))

    in_maps = []
    for k in range(NC):
        esrc = np.zeros((TPC, NIDX), dtype=np.int16)
        drel = np.full((TPC, NIDX), -1.0, dtype=np.float32)
        for t in range(TPC):
            g = k * TPC + t
            lo, hi = starts[g], starts[g + 1]
            n = hi - lo
            esrc[t, :n] = s_src[lo:hi].astype(np.int16)
            drel[t, :n] = (s_dst[lo:hi] - (g << 7)).astype(np.float32)
        # idx layout: i -> [i % 16, i // 16], replicated to 128 partitions
        esrc_dev = np.ascontiguousarray(
            np.tile(esrc.reshape(TPC, NIDX // 16, 16).transpose(0, 2, 1),
                    (1, 8, 1)).transpose(1, 0, 2).reshape(128, TPC * NIDX // 16))
        # drel layout: [128, TPC*CH], [p, t*CH + c] = drel[t, c*128+p]
        drel_dev = np.ascontiguousarray(
            drel.reshape(TPC, CH, 128).transpose(2, 0, 1).reshape(128, TPC * CH))
        sl = slice(k * NSH, (k + 1) * NSH)
        in_maps.append({
            "xT": np.ascontiguousarray(x[sl].T),
            "w1": w1, "w2": w2, "b1b": b1b, "b2b": b2b,
            "degp": np.ascontiguousarray(deg1[sl].reshape(TPC, 128).T),
            "iota": iota, "ident": ident,
            "esrc": esrc_dev, "drel": drel_dev,
        })
    return in_maps, CH


def kernel(x, edge_index, adj, W1, b1, W2, b2):
    from concourse.bass_utils import run_bass_kernel_spmd

    in_maps, CH = _host_prep(x, edge_index, W1, b1, W2, b2)
    if CH not in _CACHE:
        _CACHE[CH] = _build_program(CH)
    nc = _CACHE[CH]
    res = run_bass_kernel_spmd(nc, in_maps, list(range(NC)))
    return np.concatenate([res.results[k]["out"] for k in range(NC)], axis=0)



# revision 16
# speedup vs baseline: 1.0160x; 1.0160x over previous
"""Two-layer GCN (GCNConv -> ReLU -> GCNConv -> softmax) on 8 Trainium2 NeuronCores.

Sharding: nodes (rows of x, degrees, output) are split 2048-per-core; edges are
partitioned by destination node so each core owns the scatter for its node
shard; the small weight matrices are replicated.  Source-node features are
exchanged with an AllGather between layers.

Per-core aggregation: edges are bucketed by 128-node destination tile.  For
each tile, gathered source rows (dma_gather) are combined with a one-hot
matrix built on the vector engine (iota == dst_rel) and reduced on the tensor
engine via PSUM-accumulated matmuls; the self-loop term is an identity-matmul
chunk.  deg^-1/2 scaling, bias, relu and the row softmax run on DVE/ACT.
"""
import numpy as np

N = 16384
NC = 8
NSH = N // NC        # 2048 nodes per core
TPC = NSH // 128     # 16 destination tiles per core
C_IN, C_HID, C_OUT = 128, 64, 16

_CACHE = {}


def _build_program(CH, stage=4, reps=1, nocc=False, ablate=()):
    import concourse.bacc as bacc
    import concourse.bass as bass
    import concourse.mybir as mybir
    import concourse.tile as tile
    from concourse import library_config
    from contextlib import ExitStack

    f32 = mybir.dt.float32
    f16 = mybir.dt.float16
    i16 = mybir.dt.int16
    i32 = mybir.dt.int32
    NIDX = CH * 128

    nc = bacc.Bacc("TRN2", target_bir_lowering=False, debug=False,
                   enable_asserts=False, num_devices=NC)

    # -------- I/O --------
    d_xT = nc.dram_tensor("xT", [C_IN, NSH], f32, kind="ExternalInput").ap()
    d_w1 = nc.dram_tensor("w1", [C_IN, C_HID], f32, kind="ExternalInput").ap()
    d_w2 = nc.dram_tensor("w2", [C_HID, C_OUT], f32, kind="ExternalInput").ap()
    d_b1 = nc.dram_tensor("b1b", [128, C_HID], f32, kind="ExternalInput").ap()
    d_b2 = nc.dram_tensor("b2b", [128, C_OUT], f32, kind="ExternalInput").ap()
    d_deg = nc.dram_tensor("degp", [128, TPC], f32, kind="ExternalInput").ap()
    d_iota = nc.dram_tensor("iota", [128, 128], f32, kind="ExternalInput").ap()
    d_id = nc.dram_tensor("ident", [128, 128], f32, kind="ExternalInput").ap()
    d_esrc = nc.dram_tensor("esrc", [128, TPC * CH * 8], i16,
                            kind="ExternalInput").ap()
    d_drel = nc.dram_tensor("drel", [128, TPC * CH], f32,
                            kind="ExternalInput").ap()
    d_cnt = nc.dram_tensor("ecnt", [1, TPC], i32, kind="ExternalInput").ap()
    d_out = nc.dram_tensor("out", [NSH, C_OUT], f32, kind="ExternalOutput").ap()

    # -------- internal DRAM (collectives) --------
    ht1_sh = nc.dram_tensor("ht1_sh", [NSH, C_HID], f32).ap()
    ht1_full = nc.dram_tensor("ht1_full", [N, C_HID], f32,
                              addr_space="Shared").ap()
    ht2_sh = nc.dram_tensor("ht2_sh", [NSH, C_HID], f32).ap()
    ht2_full = nc.dram_tensor("ht2_full", [N, C_HID], f32,
                              addr_space="Shared").ap()

    rg = [list(range(NC))]

    with tile.TileContext(nc) as tc, ExitStack() as ctx:
        cp = ctx.enter_context(tc.tile_pool(name="const", bufs=1))
        gp = ctx.enter_context(tc.tile_pool(name="gather", bufs=3))
        qp = ctx.enter_context(tc.tile_pool(name="onehot", bufs=3))
        wp = ctx.enter_context(tc.tile_pool(name="work", bufs=3))
        pp = ctx.enter_context(tc.tile_pool(name="psum", bufs=3, space="PSUM"))
        pp2 = ctx.enter_context(tc.tile_pool(name="psum2", bufs=2, space="PSUM"))

        nc.gpsimd.load_library(library_config.mlp)

        # ---- persistent SBUF ----
        sb_xT = cp.tile([C_IN, NSH], f32)
        sb_w1 = cp.tile([C_IN, C_HID], f32)
        sb_w2 = cp.tile([C_HID, C_OUT], f32)
        sb_b1 = cp.tile([128, C_HID], f32)
        sb_b2 = cp.tile([128, C_OUT], f32)
        sb_deg = cp.tile([128, TPC], f32)
        sb_iota = cp.tile([128, 128], f32)
        sb_id = cp.tile([128, 128], f32)
        sb_esrc = cp.tile([128, TPC * CH * 8], i16)
        sb_drel = cp.tile([128, TPC * CH], f32)
        sb_cnt = cp.tile([1, TPC], i32)
        sb_ht1 = cp.tile([128, TPC * C_HID], f32)    # is * (x@W1), own shard
        sb_out1 = cp.tile([128, TPC * C_HID], f32)   # relu'd layer-1 out
        sb_ht2p = cp.tile([128, TPC * C_HID], f32)   # is * (out1@W2), padded
        sb_out2 = cp.tile([128, TPC * C_OUT], f32)

        for dst, src in ((sb_xT, d_xT), (sb_w1, d_w1), (sb_w2, d_w2),
                         (sb_b1, d_b1), (sb_b2, d_b2), (sb_deg, d_deg),
                         (sb_iota, d_iota), (sb_id, d_id), (sb_esrc, d_esrc),
                         (sb_drel, d_drel), (sb_cnt, d_cnt)):
            nc.sync.dma_start(dst[:], src[:])


        # trailing -1 gather indices leave their G slots unwritten; zero the
        # rotating buffers once so unwritten slots are 0.0, not NaN garbage
        for _gz in range(3):
            Gz = gp.tile([128, CH, C_HID], f32, tag="G")
            nc.vector.memset(Gz[:], 0.0)

        # ---- inv_sqrt(deg) ----
        nc.vector.memset(sb_ht2p[:], 0.0)
        sb_sq = cp.tile([128, TPC], f32)
        sb_is = cp.tile([128, TPC], f32)
        for _rep in range(reps):
          nc.scalar.sqrt(sb_sq[:], sb_deg[:])
          nc.vector.reciprocal(sb_is[:], sb_sq[:])

          # ---- ht1 = is * (x @ W1) ----
          for t in range(TPC):
              psu = pp.tile([128, C_HID], f32, tag="psA")
              nc.tensor.matmul(psu[:], sb_xT[:, t * 128:(t + 1) * 128], sb_w1[:],
                               start=True, stop=True)
              nc.vector.tensor_scalar_mul(
                  sb_ht1[:, t * C_HID:(t + 1) * C_HID], psu[:],
                  sb_is[:, t:t + 1])
          nc.sync.dma_start(
              ht1_sh.rearrange("(t p) c -> p t c", p=128),
              sb_ht1[:].rearrange("p (t c) -> p t c", t=TPC))
          if nocc:
              nc.sync.dma_start(ht1_full[0:NSH, :], ht1_sh[:])
          else:
              nc.gpsimd.collective_compute(
                  "AllGather", mybir.AluOpType.bypass, replica_groups=rg,
                  ins=[ht1_sh[:]], outs=[ht1_full[:]])

          # ---- layer 1 aggregation + fused u2 = out1 @ W2 ----
          for t in range(TPC if stage >= 1 else 0):
              G = gp.tile([128, CH, C_HID], f32, tag="G")
              if "noG" in ablate:
                  nc.sync.dma_start(G[:].rearrange("p c d -> p (c d)")[:, 0:64], ht1_full[0:128, :])
              else:
                  nc.gpsimd.dma_gather(
                      G[:], ht1_full[:],
                      sb_esrc[:, t * CH * 8:(t + 1) * CH * 8], NIDX, NIDX, C_HID,
                      single_packet=False)
              Q = qp.tile([128, CH * 128], f16, tag="Q")
              if "noQ" in ablate:
                  nc.vector.memset(Q[:, 0:128], 0.0)
              else:
                  nc.vector.tensor_tensor(
                  Q[:].rearrange("p (c d) -> p c d", c=CH),
                  sb_drel[:, t * CH:(t + 1) * CH].unsqueeze(2)
                      .broadcast_to([128, CH, 128]),
                  sb_iota[:].unsqueeze(1).broadcast_to([128, CH, 128]),
                  mybir.AluOpType.is_equal)
              Gh = gp.tile([128, CH, C_HID], f16, tag="Gh")
              nc.scalar.activation(Gh[:], G[:],
                                   mybir.ActivationFunctionType.Copy)
              ps = pp.tile([128, C_HID], f32, tag="psA")
              for c in range(1 if "noMM" in ablate else CH):
                  nc.tensor.matmul(ps[:], Q[:, c * 128:(c + 1) * 128],
                                   Gh[:, c, :], start=(c == 0), stop=False)
              nc.tensor.matmul(ps[:], sb_id[:],
                               sb_ht1[:, t * C_HID:(t + 1) * C_HID],
                               start=False, stop=True)
              o1 = sb_out1[:, t * C_HID:(t + 1) * C_HID]
              nc.vector.scalar_tensor_tensor(
                  o1, ps[:], sb_is[:, t:t + 1], sb_b1[:],
                  mybir.AluOpType.mult, mybir.AluOpType.add)
              nc.scalar.activation(o1, o1, mybir.ActivationFunctionType.Relu)

              # u2 tile: transpose out1 tile, matmul with W2, scale by is
              pstr = pp2.tile([C_HID, 128], f32, tag="psT")
              nc.tensor.transpose(pstr[:], o1, sb_id[:])
              o1T = wp.tile([C_HID, 128], f32, tag="o1T")
              nc.vector.tensor_copy(o1T[:], pstr[:])
              psu2 = pp2.tile([128, C_OUT], f32, tag="psU")
              nc.tensor.matmul(psu2[:], o1T[:], sb_w2[:], start=True, stop=True)
              nc.vector.tensor_scalar_mul(
                  sb_ht2p[:, t * C_HID:t * C_HID + C_OUT], psu2[:],
                  sb_is[:, t:t + 1])

          if stage >= 2:
              nc.sync.dma_start(
                  ht2_sh.rearrange("(t p) c -> p t c", p=128),
                  sb_ht2p[:].rearrange("p (t c) -> p t c", t=TPC))
              if nocc:
                  nc.sync.dma_start(ht2_full[0:NSH, :], ht2_sh[:])
              else:
                  nc.gpsimd.collective_compute(
                      "AllGather", mybir.AluOpType.bypass, replica_groups=rg,
                      ins=[ht2_sh[:]], outs=[ht2_full[:]])

          # ---- layer 2 aggregation + softmax ----
          nc.vector.memset(sb_out2[:], 0.0)
          for t in range(TPC if stage >= 3 else 0):
              G = gp.tile([128, CH, C_HID], f32, tag="G")
              nc.gpsimd.dma_gather(
                  G[:], ht2_full[:],
                  sb_esrc[:, t * CH * 8:(t + 1) * CH * 8], NIDX, NIDX, C_HID,
                  single_packet=False)
              Q = qp.tile([128, CH * 128], f16, tag="Q")
              nc.vector.tensor_tensor(
                  Q[:].rearrange("p (c d) -> p c d", c=CH),
                  sb_drel[:, t * CH:(t + 1) * CH].unsqueeze(2)
                      .broadcast_to([128, CH, 128]),
                  sb_iota[:].unsqueeze(1).broadcast_to([128, CH, 128]),
                  mybir.AluOpType.is_equal)
              Gh = gp.tile([128, CH, C_OUT], f16, tag="Gh2")
              nc.scalar.activation(Gh[:], G[:, :, 0:C_OUT],
                                   mybir.ActivationFunctionType.Copy)
              ps = pp.tile([128, C_OUT], f32, tag="psA")
              for c in range(CH):
                  nc.tensor.matmul(ps[:], Q[:, c * 128:(c + 1) * 128],
                                   Gh[:, c, :], start=(c == 0), stop=False)
              nc.tensor.matmul(ps[:], sb_id[:],
                               sb_ht2p[:, t * C_HID:t * C_HID + C_OUT],
                               start=False, stop=True)
              z = wp.tile([128, C_OUT], f32, tag="z")
              nc.vector.scalar_tensor_tensor(
                  z[:], ps[:], sb_is[:, t:t + 1], sb_b2[:],
                  mybir.AluOpType.mult, mybir.AluOpType.add)
              # softmax along the 16 columns
              negm = wp.tile([128, 1], f32, tag="negm")
              nc.vector.tensor_reduce(negm[:], z[:], mybir.AxisListType.X,
                                      mybir.AluOpType.max, negate=True)
              e = sb_out2[:, t * C_OUT:(t + 1) * C_OUT]
              nc.scalar.activation(e, z[:], mybir.ActivationFunctionType.Exp,
                                   bias=negm[:, 0:1], scale=1.0)
              ssum = wp.tile([128, 1], f32, tag="ssum")
              nc.vector.tensor_reduce(ssum[:], e, mybir.AxisListType.X,
                                      mybir.AluOpType.add)
              rcp = wp.tile([128, 1], f32, tag="rcp")
              nc.vector.reciprocal(rcp[:], ssum[:])
              nc.vector.tensor_scalar_mul(e, e, rcp[:, 0:1])

          nc.sync.dma_start(
              d_out.rearrange("(t p) c -> p t c", p=128),
              sb_out2[:].rearrange("p (t c) -> p t c", t=TPC))

    nc.compile()
    return nc


def _host_prep(x, edge_index, W1, b1, W2, b2):
    src = np.asarray(edge_index[0]).astype(np.int64)
    dst = np.asarray(edge_index[1]).astype(np.int64)
    x = np.asarray(x, dtype=np.float32)

    deg1 = (np.bincount(dst, minlength=N) + 1).astype(np.float32)
    tile_id = dst >> 7
    order = np.argsort(tile_id, kind="stable")
    s_src = src[order]
    s_tile = tile_id[order]
    counts = np.bincount(s_tile, minlength=128)
    CH = int(np.ceil(counts.max() / 128))
    NIDX = CH * 128
    starts = np.zeros(129, dtype=np.int64)
    np.cumsum(counts, out=starts[1:])
    s_dst = dst[order]

    iota = np.ascontiguousarray(
        np.tile(np.arange(128, dtype=np.float32), (128, 1)))
    ident = np.eye(128, dtype=np.float32)
    b1b = np.ascontiguousarray(np.tile(np.asarray(b1, np.float32), (128, 1)))
    b2b = np.ascontiguousarray(np.tile(np.asarray(b2, np.float32), (128, 1)))
    w1 = np.ascontiguousarray(np.asarray(W1, np.float32))
    w2 = np.ascontiguousarray(np.asarray(W2, np.float32))

    in_maps = []
    for k in range(NC):
        esrc = np.zeros((TPC, NIDX), dtype=np.int16)
        drel = np.full((TPC, NIDX), -1.0, dtype=np.float32)
        ecnt = np.full((1, TPC), NIDX, dtype=np.int32)
        for t in range(TPC):
            g = k * TPC + t
            lo, hi = starts[g], starts[g + 1]
            n = hi - lo
            esrc[t, :n] = s_src[lo:hi].astype(np.int16)
            drel[t, :n] = (s_dst[lo:hi] - (g << 7)).astype(np.float32)
        # idx layout: i -> [i % 16, i // 16], replicated to 128 partitions
        esrc_dev = np.ascontiguousarray(
            np.tile(esrc.reshape(TPC, NIDX // 16, 16).transpose(0, 2, 1),
                    (1, 8, 1)).transpose(1, 0, 2).reshape(128, TPC * NIDX // 16))
        # drel layout: [128, TPC*CH], [p, t*CH + c] = drel[t, c*128+p]
        drel_dev = np.ascontiguousarray(
            drel.reshape(TPC, CH, 128).transpose(2, 0, 1).reshape(128, TPC * CH))
        sl = slice(k * NSH, (k + 1) * NSH)
        in_maps.append({
            "xT": np.ascontiguousarray(x[sl].T),
            "w1": w1, "w2": w2, "b1b": b1b, "b2b": b2b,
            "degp": np.ascontiguousarray(deg1[sl].reshape(TPC, 128).T),
            "iota": iota, "ident": ident,
            "esrc": esrc_dev, "drel": drel_dev, "ecnt": ecnt,
        })
    return in_maps, CH


def kernel(x, edge_index, adj, W1, b1, W2, b2):
    from concourse.bass_utils import run_bass_kernel_spmd

    in_maps, CH = _host_prep(x, edge_index, W1, b1, W2, b2)
    if CH not in _CACHE:
        _CACHE[CH] = _build_program(CH)
    nc = _CACHE[CH]
    res = run_bass_kernel_spmd(nc, in_maps, list(range(NC)))
    return np.concatenate([res.results[k]["out"] for k in range(NC)], axis=0)

